# revision 14
# baseline (speedup 1.0000x reference)
"""Trainium2 Bass kernel for nn_H_DYNA_42348377538865 (scatter_memory GRU + memory attention).

Self-contained: shards node dim N=512 across 8 NeuronCores (64 nodes/core),
fully-unrolled 24-step recurrence per core, gathers on host.

v6 design (vs baseline 633798ns):
  - pair-packed layout: 2 column-chunks of 512 stacked on 128 partitions
    (top rows 0:64 = even chunk, bottom 64:128 = odd chunk) for h/gates;
    halves elementwise instruction cost on ACT/DVE/Pool.
  - sigmoid via tanh identity (sigmoid(x) = 0.5 + 0.5*tanh(x/2), weights
    pre-halved): every ACT func is in the `exp_and_others` table set ->
    zero LoadActFuncSet reloads (was 128 x 1283ns).
  - block-diagonal [128,128] gate weights: one matmul per gate per pair.
  - decode steps t>=13 fold x = y_prev = Wo^T h + bo into the gate h-weights
    (exact algebra) -> no x matmuls, no y feedback copies.
  - encode x enters via rank-2 matmuls against a [2, 1024] x tile.
  - q-cache: one block-diag matmul per pair -> [64,512] PSUM, partition-shift
    copies into the rolling qb slot tiles.
  - engine balance: ACT = exp/tanh/copies, DVE = recip/fn/affine/mul/copy,
    Pool (no PSUM access) = fp16 SBUF sub/add of the GRU update.
"""
import numpy as np
import sys

for _p in ("/opt/trn_rl_repo",):
    if _p not in sys.path:
        sys.path.append(_p)

import concourse.bass as bass
import concourse.bacc as bacc
import concourse.mybir as mybir
import concourse.tile as tile
from concourse import bass_utils
from concourse.alu_op_type import AluOpType

B, T, HORIZON, N = 32, 12, 12, 512
IN, OUT, H, P = 1, 1, 64, 32
S, ML, MG, DE = 12, 64, 32, 10
NCORES = 8
NL = N // NCORES        # 64 nodes/core
NB = NL * B             # 2048 cols/core
NSTEP = T + HORIZON     # 24
NPAIR = 2               # two 1024-col pairs; each pair = 2 chunks of 512

F32 = mybir.dt.float32
BF16 = mybir.dt.bfloat16
FP16 = mybir.dt.float16
AF = mybir.ActivationFunctionType
OP = AluOpType


def build_nc():
    nc = bacc.Bacc("TRN2", target_bir_lowering=False, debug=False)
    d = {}
    def din(name, shape, dt=FP16):
        d[name] = nc.dram_tensor(name, shape, dt, kind="ExternalInput")
    din("xsrc", [T * 2, NB // 2])
    din("msk", [128, S * 3 * 96])
    din("nsw2", [128, NL * 64])
    din("fmean", [96, 64], BF16)
    din("fsum", [96, 64], BF16)
    din("zbd", [128, 128]); din("rbd", [128, 128])
    din("zbd_dec", [128, 128]); din("rbd_dec", [128, 128])
    din("cbd05", [128, 128]); din("cbd_dec", [128, 128])
    din("xzw", [2, 128]); din("xrw", [2, 128]); din("xcw", [2, 128])
    din("qw128", [128, 64]); din("ow128", [128, 2])
    din("lbias", [96, 1], F32)
    din("bz05", [128, 1], F32); din("br05", [128, 1], F32)
    din("bz_dec", [128, 1], F32); din("br_dec", [128, 1], F32)
    din("bc2", [128, 1], F32); din("bc_dec", [128, 1], F32)
    din("bo2", [2, 1], F32)
    ys_d = nc.dram_tensor("ys", [HORIZON * 2, NB // 2], FP16, kind="ExternalOutput")

    with tile.TileContext(nc) as tc:
        with (
            tc.tile_pool(name="consts", bufs=1) as cp,
            tc.tile_pool(name="sp", bufs=3) as sp,
            tc.tile_pool(name="pp_lg", bufs=1, space="PSUM") as pp_lg,
            tc.tile_pool(name="pp_fu", bufs=1, space="PSUM") as pp_fu,
            tc.tile_pool(name="pp_su", bufs=1, space="PSUM") as pp_su,
            tc.tile_pool(name="pp_z", bufs=1, space="PSUM") as pp_z,
            tc.tile_pool(name="pp_r", bufs=1, space="PSUM") as pp_r,
            tc.tile_pool(name="pp_acc", bufs=1, space="PSUM") as pp_acc,
            tc.tile_pool(name="pp_qy", bufs=1, space="PSUM") as pp_qy,
        ):
            # ---- constants
            c = {}
            for name, shape, dt in [
                ("msk", [128, S * 3 * 96], FP16), ("nsw2", [128, NL * 64], FP16),
                ("fmean", [96, 64], BF16), ("fsum", [96, 64], BF16),
                ("zbd", [128, 128], FP16), ("rbd", [128, 128], FP16),
                ("zbd_dec", [128, 128], FP16), ("rbd_dec", [128, 128], FP16),
                ("cbd05", [128, 128], FP16), ("cbd_dec", [128, 128], FP16),
                ("xzw", [2, 128], FP16), ("xrw", [2, 128], FP16), ("xcw", [2, 128], FP16),
                ("qw128", [128, 64], FP16), ("ow128", [128, 2], FP16),
                ("lbias", [96, 1], F32),
                ("bz05", [128, 1], F32), ("br05", [128, 1], F32),
                ("bz_dec", [128, 1], F32), ("br_dec", [128, 1], F32),
                ("bc2", [128, 1], F32), ("bc_dec", [128, 1], F32),
                ("bo2", [2, 1], F32),
            ]:
                c[name] = cp.tile(shape, dt, name=name)
                nc.sync.dma_start(c[name][:], d[name].ap())

            # ---- state
            # q cache: 3 tiles of [128 = 4 slots x 32 P, NB real cols].
            # Stores q-without-bias (bq folded into the logit bias `lbias`,
            # exact because the slot->slice rotation is a permutation), so
            # empty slots are simply zero.
            qb = []
            for g in range(3):
                q = cp.tile([128, NB], FP16, name=f"qb{g}")
                nc.vector.memset(q[:], 0.0)
                qb.append(q)
            # pair-packed hidden state
            h2 = cp.tile([128, NB // 2], FP16, name="h2")
            nc.vector.memset(h2[:], 0.0)
            # x input staging (double-buffered over steps)
            xr = [cp.tile([2, NB // 2], FP16, name=f"xr{i}") for i in range(2)]
            nc.sync.dma_start(xr[0][:], d["xsrc"][0:2, :])
            ystage = cp.tile([2, NB // 2], FP16, name="ystage")

            for t in range(NSTEP):
                r = t % S
                j = t % S
                g_w, row_w = j // 4, (j % 4) * 32
                enc = t <= T                  # t<=12: x comes from inputs
                dec_w = t > T                 # t>12: x folded into weights
                zbd_t = c["zbd_dec"] if dec_w else c["zbd"]
                rbd_t = c["rbd_dec"] if dec_w else c["rbd"]
                cbd_t = c["cbd_dec"] if dec_w else c["cbd05"]
                bz_t = c["bz_dec"] if dec_w else c["bz05"]
                br_t = c["br_dec"] if dec_w else c["br05"]
                bc_t = c["bc_dec"] if dec_w else c["bc2"]
                xr_t = xr[min(t, T - 1) % 2]

                for p in range(NPAIR):
                    pc = slice(p * 512, (p + 1) * 512)        # packed cols
                    tc_cols = slice(p * 1024, p * 1024 + 512)  # real cols top
                    bc_cols = slice(p * 1024 + 512, (p + 1) * 1024)  # real bottom

                    # ---- attention logits for both chunks of the pair
                    lg2 = pp_lg.tile([96, 1024], F32, tag="lg")
                    for half, rcols in ((0, tc_cols), (1, bc_cols)):
                        hs = slice(half * 512, (half + 1) * 512)
                        for g in range(3):
                            off = (r * 3 + g) * 96
                            nc.tensor.matmul(
                                lg2[:, hs], c["msk"][:, off:off + 96], qb[g][:, rcols],
                                start=(g == 0), stop=(g == 2),
                            )
                    ex2 = sp.tile([96, 1024], BF16, tag="ex")
                    nc.scalar.activation(ex2[:], lg2[:], AF.Exp, bias=c["lbias"][:, 0:1])
                    fu2 = pp_fu.tile([128, 512], F32, tag="fu")
                    su2 = pp_su.tile([128, 512], F32, tag="su")
                    nc.tensor.matmul(fu2[0:64, :], c["fmean"][:], ex2[:, 0:512],
                                     start=True, stop=True, tile_position=(0, 0),
                                     skip_group_check=True)
                    nc.tensor.matmul(fu2[64:128, :], c["fmean"][:], ex2[:, 512:1024],
                                     start=True, stop=True, tile_position=(0, 64),
                                     skip_group_check=True)
                    nc.tensor.matmul(su2[0:64, :], c["fsum"][:], ex2[:, 0:512],
                                     start=True, stop=True, tile_position=(0, 0),
                                     skip_group_check=True)
                    nc.tensor.matmul(su2[64:128, :], c["fsum"][:], ex2[:, 512:1024],
                                     start=True, stop=True, tile_position=(0, 64),
                                     skip_group_check=True)
                    rt2 = sp.tile([128, 512], F32, tag="rt")
                    nc.vector.reciprocal_approx_fast(rt2[:], su2[:])
                    fn2 = sp.tile([128, 512], FP16, tag="fn")
                    nc.vector.tensor_mul(fn2[:], fu2[:], rt2[:])

                    # ---- gates z, r (pair-packed, block-diag weights)
                    z2 = pp_z.tile([128, 512], F32, tag="z2")
                    r2 = pp_r.tile([128, 512], F32, tag="r2")
                    nc.tensor.matmul(z2[:], zbd_t[:], h2[:, pc],
                                     start=True, stop=not enc, skip_group_check=True)
                    if enc:
                        nc.tensor.matmul(z2[:], c["xzw"][:], xr_t[:, pc],
                                         start=False, stop=True, tile_position=(0, 0),
                                         skip_group_check=True)
                    nc.tensor.matmul(r2[:], rbd_t[:], h2[:, pc],
                                     start=True, stop=not enc, skip_group_check=True)
                    if enc:
                        nc.tensor.matmul(r2[:], c["xrw"][:], xr_t[:, pc],
                                         start=False, stop=True, tile_position=(0, 0),
                                         skip_group_check=True)
                    tzr2 = sp.tile([128, 1024], FP16, tag="tzr")
                    nc.scalar.activation(tzr2[:, 0:512], z2[:], AF.Tanh, bias=bz_t[:, 0:1])
                    nc.scalar.activation(tzr2[:, 512:1024], r2[:], AF.Tanh, bias=br_t[:, 0:1])
                    # z gate: sigmoid(x) = 0.5 + 0.5*tanh(x/2) (weights pre-halved)
                    zs2 = sp.tile([128, 512], FP16, tag="zs")
                    nc.vector.tensor_scalar(zs2[:], tzr2[:, 0:512], 0.5, 0.5, OP.mult, OP.add)
                    # r gate folded into cw weights: Wch^T(r.h) = (.5Wch)^T h + (.5Wch)^T (tr.h)
                    trh2 = sp.tile([128, 512], FP16, tag="trh")
                    nc.vector.tensor_mul(trh2[:], tzr2[:, 512:1024], h2[:, pc])

                    # ---- candidate hc
                    acc2 = pp_acc.tile([128, 512], F32, tag="acc")
                    nc.tensor.matmul(acc2[:], cbd_t[:], h2[:, pc],
                                     start=True, stop=False, skip_group_check=True)
                    nc.tensor.matmul(acc2[:], c["cbd05"][:], trh2[:],
                                     start=False, stop=False, skip_group_check=True)
                    if enc:
                        nc.tensor.matmul(acc2[:], c["xcw"][:], xr_t[:, pc],
                                         start=False, stop=False, tile_position=(0, 0),
                                         skip_group_check=True)
                    for k in range(16):
                        n = 32 * p + k
                        nc.tensor.matmul(
                            acc2[0:64, k * 32:(k + 1) * 32],
                            c["nsw2"][0:64, n * 64:(n + 1) * 64],
                            fn2[0:64, k * 32:(k + 1) * 32],
                            start=False, stop=False, tile_position=(0, 0),
                            skip_group_check=True,
                        )
                    for k in range(16):
                        n = 32 * p + 16 + k
                        nc.tensor.matmul(
                            acc2[64:128, k * 32:(k + 1) * 32],
                            c["nsw2"][64:128, n * 64:(n + 1) * 64],
                            fn2[64:128, k * 32:(k + 1) * 32],
                            start=False, stop=(k == 15), tile_position=(64, 64),
                            skip_group_check=True,
                        )
                    hc2 = sp.tile([128, 512], FP16, tag="hc")
                    nc.scalar.activation(hc2[:], acc2[:], AF.Tanh, bias=bc_t[:, 0:1])

                    # ---- GRU update: h += (0.5 + 0.5 tz) * (hc - h)
                    d2 = sp.tile([128, 512], FP16, tag="d2")
                    nc.gpsimd.tensor_sub(d2[:], hc2[:], h2[:, pc])
                    nc.vector.tensor_mul(d2[:], zs2[:], d2[:])
                    nc.gpsimd.tensor_add(h2[:, pc], h2[:, pc], d2[:])

                    # ---- q cache update (one block-diag matmul + 2 shift copies)
                    qp2 = pp_qy.tile([64, 512], F32, tag="qy")
                    nc.tensor.matmul(qp2[:], c["qw128"][:], h2[:, pc],
                                     start=True, stop=True, skip_group_check=True)
                    nc.vector.tensor_copy(qb[g_w][row_w:row_w + 32, tc_cols], qp2[0:32, :])
                    nc.scalar.activation(qb[g_w][row_w:row_w + 32, bc_cols], qp2[32:64, :],
                                         AF.Identity)

                    # ---- decode output
                    if t >= T:
                        y2 = pp_qy.tile([2, 512], F32, tag="qy")
                        nc.tensor.matmul(y2[:], c["ow128"][:], h2[:, pc],
                                         start=True, stop=True, skip_group_check=True)
                        nc.scalar.activation(ystage[:, pc], y2[:], AF.Identity,
                                             bias=c["bo2"][:, 0:1])

                # ---- step epilogue DMAs
                if t < T - 1:
                    nc.sync.dma_start(xr[(t + 1) % 2][:],
                                      d["xsrc"][2 * (t + 1):2 * (t + 2), :])
                if t >= T:
                    dstep = t - T
                    nc.sync.dma_start(ys_d[2 * dstep:2 * (dstep + 1), :], ystage[:])
    nc.compile()
    return nc


def precompute(inp):
    f32 = np.float32
    lm = np.asarray(inp["local_mem"], f32)
    gm = np.asarray(inp["global_mem"], f32)
    Wq = np.asarray(inp["Wq"], f32)
    bq = np.asarray(inp["bq"], f32)
    node_emb = np.asarray(inp["node_emb"], f32)
    wp = np.asarray(inp["weight_pool"], f32)
    Wz = np.asarray(inp["Wz"], f32); bz = np.asarray(inp["bz"], f32)
    Wr = np.asarray(inp["Wr"], f32); br = np.asarray(inp["br"], f32)
    Wc = np.asarray(inp["Wc"], f32); bc = np.asarray(inp["bc"], f32)
    Wo = np.asarray(inp["Wo"], f32); bo = np.asarray(inp["bo"], f32)

    c = {}
    # per-node hypernet weights, duplicated on both partition halves
    c["nsw_full"] = np.einsum("nd,dfh->nfh", node_emb, wp).astype(f32)  # [N,2P,H]
    # memory rotation stacks (identical to baseline)
    memsl = np.concatenate([lm.transpose(2, 0, 1), gm.transpose(2, 0, 1)], axis=1)  # [P,96,S]
    ms = np.zeros((128, S, 3, 96), f32)
    for rr in range(S):
        for g in range(3):
            for i in range(4):
                s = (4 * g + i - rr) % S
                ms[32 * i:32 * (i + 1), rr, g, :] = memsl[:, :, s]
    c["msk"] = ms.reshape(128, S * 3 * 96)
    lmean, gmean = lm.mean(axis=1), gm.mean(axis=1)
    fm = np.zeros((96, 64), f32)
    fm[:ML, :P] = lmean
    fm[ML:, P:2 * P] = gmean
    c["fmean"] = fm
    fs = np.zeros((96, 64), f32)
    fs[:ML, :P] = 1.0
    fs[ML:, P:2 * P] = 1.0
    c["fsum"] = fs

    def blockdiag(w):  # w [64,64] as lhsT [k,i] -> [128,128]
        o = np.zeros((128, 128), f32)
        o[0:64, 0:64] = w
        o[64:128, 64:128] = w
        return o

    Wzh, Wzx = Wz[1:], Wz[0]   # [64,64], [64]
    Wrh, Wrx = Wr[1:], Wr[0]
    Wch, Wcx = Wc[1:], Wc[0]
    wo = Wo[:, 0]              # [64]
    c["zbd"] = blockdiag(0.5 * Wzh)
    c["rbd"] = blockdiag(0.5 * Wrh)
    c["cbd05"] = blockdiag(0.5 * Wch)
    c["zbd_dec"] = blockdiag(0.5 * (Wzh + np.outer(wo, Wzx)))
    c["rbd_dec"] = blockdiag(0.5 * (Wrh + np.outer(wo, Wrx)))
    c["cbd_dec"] = blockdiag(0.5 * Wch + np.outer(wo, Wcx))

    def xw2(wx, scale):  # [2,128] rank-2 x weights
        o = np.zeros((2, 128), f32)
        o[0, 0:64] = scale * wx
        o[1, 64:128] = scale * wx
        return o

    c["xzw"] = xw2(Wzx, 0.5)
    c["xrw"] = xw2(Wrx, 0.5)
    c["xcw"] = xw2(Wcx, 1.0)

    q128 = np.zeros((128, 64), f32)
    q128[0:64, 0:32] = Wq
    q128[64:128, 32:64] = Wq
    c["qw128"] = q128
    o128 = np.zeros((128, 2), f32)
    o128[0:64, 0] = wo
    o128[64:128, 1] = wo
    c["ow128"] = o128

    # bq folded into a constant logit bias: lbias[m] = bq . sum_s mem[m, s, :]
    lb = np.zeros((96, 1), f32)
    lb[:ML, 0] = np.einsum("msp,p->m", lm, bq)
    lb[ML:, 0] = np.einsum("msp,p->m", gm, bq)
    c["lbias"] = lb
    c["bz05"] = np.tile(0.5 * bz, 2).reshape(128, 1)
    c["br05"] = np.tile(0.5 * br, 2).reshape(128, 1)
    c["bz_dec"] = np.tile(0.5 * (bz + Wzx * bo[0]), 2).reshape(128, 1)
    c["br_dec"] = np.tile(0.5 * (br + Wrx * bo[0]), 2).reshape(128, 1)
    c["bc2"] = np.tile(bc, 2).reshape(128, 1)
    c["bc_dec"] = np.tile(bc + Wcx * bo[0], 2).reshape(128, 1)
    c["bo2"] = np.full((2, 1), bo[0], f32)
    return c


def _f16(a):
    return np.ascontiguousarray(a).astype(np.float16)


def _bf16(a):
    import ml_dtypes
    return np.ascontiguousarray(a).astype(ml_dtypes.bfloat16)


def make_in_maps(inp):
    c = precompute(inp)
    src = np.asarray(inp["source"], np.float32)
    shared = {
        "msk": _f16(c["msk"]),
        "fmean": _bf16(c["fmean"]), "fsum": _bf16(c["fsum"]),
        "zbd": _f16(c["zbd"]), "rbd": _f16(c["rbd"]),
        "zbd_dec": _f16(c["zbd_dec"]), "rbd_dec": _f16(c["rbd_dec"]),
        "cbd05": _f16(c["cbd05"]), "cbd_dec": _f16(c["cbd_dec"]),
        "xzw": _f16(c["xzw"]), "xrw": _f16(c["xrw"]), "xcw": _f16(c["xcw"]),
        "qw128": _f16(c["qw128"]), "ow128": _f16(c["ow128"]),
        "lbias": c["lbias"], "bz05": c["bz05"], "br05": c["br05"],
        "bz_dec": c["bz_dec"], "br_dec": c["br_dec"],
        "bc2": c["bc2"], "bc_dec": c["bc_dec"], "bo2": c["bo2"],
    }
    in_maps = []
    for core in range(NCORES):
        nodes = slice(core * NL, (core + 1) * NL)
        # real col = n_local*32 + b ; xsrc packed [T, 2, 1024]:
        # [t, half, p*512 + j] = x_t[real col p*1024 + half*512 + j]
        xs = src[:, :, nodes, 0].transpose(1, 2, 0).reshape(T, NB)  # [T, real]
        xs = xs.reshape(T, 2, 2, 512).transpose(0, 2, 1, 3).reshape(T * 2, NB // 2)
        nswc = c["nsw_full"][nodes].transpose(1, 0, 2).reshape(64, NL * 64)  # [f, n*64+h]
        nsw2 = np.concatenate([nswc, nswc], axis=0)  # [128, NL*64]
        in_maps.append(dict(shared, xsrc=_f16(xs), nsw2=_f16(nsw2)))
    return in_maps


def assemble(results):
    out = np.zeros((B, HORIZON, N, OUT), np.float32)
    for core in range(NCORES):
        nodes = slice(core * NL, (core + 1) * NL)
        ys = np.asarray(results[core]["ys"], np.float32)  # [HORIZON, 2, 1024]
        # real col = p*1024 + half*512 + j
        ysr = ys.reshape(HORIZON, 2, 2, 512).transpose(0, 2, 1, 3).reshape(HORIZON, NB)
        out[:, :, nodes, 0] = ysr.reshape(HORIZON, NL, B).transpose(2, 0, 1)
    return out


_NC_CACHE = {}


def kernel(**inputs):
    if "nc" not in _NC_CACHE:
        _NC_CACHE["nc"] = build_nc()
    nc = _NC_CACHE["nc"]
    in_maps = make_in_maps(inputs)
    res = bass_utils.run_bass_kernel_spmd(nc, in_maps, core_ids=list(range(NCORES)))
    return assemble(res.results)


# revision 22
# speedup vs baseline: 1.1768x; 1.1768x over previous
"""Trainium2 Bass kernel for nn_H_DYNA_42348377538865 (scatter_memory GRU + memory attention).

Self-contained: shards node dim N=512 across 8 NeuronCores (64 nodes/core),
fully-unrolled 24-step recurrence per core, gathers on host.

v6 design (vs baseline 633798ns):
  - pair-packed layout: 2 column-chunks of 512 stacked on 128 partitions
    (top rows 0:64 = even chunk, bottom 64:128 = odd chunk) for h/gates;
    halves elementwise instruction cost on ACT/DVE/Pool.
  - sigmoid via tanh identity (sigmoid(x) = 0.5 + 0.5*tanh(x/2), weights
    pre-halved): every ACT func is in the `exp_and_others` table set ->
    zero LoadActFuncSet reloads (was 128 x 1283ns).
  - block-diagonal [128,128] gate weights: one matmul per gate per pair.
  - decode steps t>=13 fold x = y_prev = Wo^T h + bo into the gate h-weights
    (exact algebra) -> no x matmuls, no y feedback copies.
  - encode x enters via rank-2 matmuls against a [2, 1024] x tile.
  - q-cache: one block-diag matmul per pair -> [64,512] PSUM, partition-shift
    copies into the rolling qb slot tiles.
  - engine balance: ACT = exp/tanh/copies, DVE = recip/fn/affine/mul/copy,
    Pool (no PSUM access) = fp16 SBUF sub/add of the GRU update.
"""
import numpy as np
import sys

for _p in ("/opt/trn_rl_repo",):
    if _p not in sys.path:
        sys.path.append(_p)

import concourse.bass as bass
import concourse.bacc as bacc
import concourse.mybir as mybir
import concourse.tile as tile
from concourse import bass_utils
from concourse.alu_op_type import AluOpType

B, T, HORIZON, N = 32, 12, 12, 512
IN, OUT, H, P = 1, 1, 64, 32
S, ML, MG, DE = 12, 64, 32, 10
NCORES = 8
NL = N // NCORES        # 64 nodes/core
NB = NL * B             # 2048 cols/core
NSTEP = T + HORIZON     # 24
NPAIR = 2               # two 1024-col pairs; each pair = 2 chunks of 512

F32 = mybir.dt.float32
BF16 = mybir.dt.bfloat16
FP16 = mybir.dt.float16
AF = mybir.ActivationFunctionType
OP = AluOpType


def build_nc():
    nc = bacc.Bacc("TRN2", target_bir_lowering=False, debug=False)
    d = {}
    def din(name, shape, dt=FP16):
        d[name] = nc.dram_tensor(name, shape, dt, kind="ExternalInput")
    din("xsrc", [T * 2, NB // 2])
    din("msk", [128, S * 3 * 96])
    din("nswbd", [128, (NL // 2) * 128])
    din("fmean", [96, 64], BF16)
    din("fsum", [96, 64], BF16)
    din("zbd", [128, 128]); din("rbd", [128, 128])
    din("zbd_dec", [128, 128]); din("rbd_dec", [128, 128])
    din("cbd05", [128, 128]); din("cbd_dec", [128, 128])
    din("xzw", [2, 128]); din("xrw", [2, 128]); din("xcw", [2, 128])
    din("qw128", [128, 64]); din("ow128", [128, 2])
    din("lbias", [96, 1], F32)
    din("bz05", [128, 1], F32); din("br05", [128, 1], F32)
    din("bz_dec", [128, 1], F32); din("br_dec", [128, 1], F32)
    din("bc2", [128, 1], F32); din("bc_dec", [128, 1], F32)
    din("bo2", [2, 1], F32)
    ys_d = nc.dram_tensor("ys", [HORIZON * 2, NB // 2], FP16, kind="ExternalOutput")

    with tile.TileContext(nc) as tc:
        with (
            tc.tile_pool(name="consts", bufs=1) as cp,
            tc.tile_pool(name="sp", bufs=3) as sp,
            tc.tile_pool(name="pp_lg", bufs=1, space="PSUM") as pp_lg,
            tc.tile_pool(name="pp_fu", bufs=1, space="PSUM") as pp_fu,
            tc.tile_pool(name="pp_su", bufs=1, space="PSUM") as pp_su,
            tc.tile_pool(name="pp_z", bufs=1, space="PSUM") as pp_z,
            tc.tile_pool(name="pp_r", bufs=1, space="PSUM") as pp_r,
            tc.tile_pool(name="pp_acc", bufs=1, space="PSUM") as pp_acc,
            tc.tile_pool(name="pp_qy", bufs=1, space="PSUM") as pp_qy,
        ):
            # ---- constants
            c = {}
            for name, shape, dt in [
                ("msk", [128, S * 3 * 96], FP16), ("nswbd", [128, (NL // 2) * 128], FP16),
                ("fmean", [96, 64], BF16), ("fsum", [96, 64], BF16),
                ("zbd", [128, 128], FP16), ("rbd", [128, 128], FP16),
                ("zbd_dec", [128, 128], FP16), ("rbd_dec", [128, 128], FP16),
                ("cbd05", [128, 128], FP16), ("cbd_dec", [128, 128], FP16),
                ("xzw", [2, 128], FP16), ("xrw", [2, 128], FP16), ("xcw", [2, 128], FP16),
                ("qw128", [128, 64], FP16), ("ow128", [128, 2], FP16),
                ("lbias", [96, 1], F32),
                ("bz05", [128, 1], F32), ("br05", [128, 1], F32),
                ("bz_dec", [128, 1], F32), ("br_dec", [128, 1], F32),
                ("bc2", [128, 1], F32), ("bc_dec", [128, 1], F32),
                ("bo2", [2, 1], F32),
            ]:
                c[name] = cp.tile(shape, dt, name=name)
                nc.sync.dma_start(c[name][:], d[name].ap())

            # ---- state
            # q cache: 3 tiles of [128 = 4 slots x 32 P, NB real cols].
            # Stores q-without-bias (bq folded into the logit bias `lbias`,
            # exact because the slot->slice rotation is a permutation), so
            # empty slots are simply zero.
            qb = []
            for g in range(3):
                q = cp.tile([128, NB], FP16, name=f"qb{g}")
                nc.vector.memset(q[:], 0.0)
                qb.append(q)
            # pair-packed hidden state
            h2 = cp.tile([128, NB // 2], FP16, name="h2")
            nc.vector.memset(h2[:], 0.0)
            # x input staging (double-buffered over steps)
            xr = [cp.tile([2, NB // 2], FP16, name=f"xr{i}") for i in range(2)]
            nc.sync.dma_start(xr[0][:], d["xsrc"][0:2, :])
            ystage = cp.tile([2, NB // 2], FP16, name="ystage")

            for t in range(NSTEP):
                r = t % S
                j = t % S
                g_w, row_w = j // 4, (j % 4) * 32
                enc = t <= T                  # t<=12: x comes from inputs
                dec_w = t > T                 # t>12: x folded into weights
                zbd_t = c["zbd_dec"] if dec_w else c["zbd"]
                rbd_t = c["rbd_dec"] if dec_w else c["rbd"]
                cbd_t = c["cbd_dec"] if dec_w else c["cbd05"]
                bz_t = c["bz_dec"] if dec_w else c["bz05"]
                br_t = c["br_dec"] if dec_w else c["br05"]
                bc_t = c["bc_dec"] if dec_w else c["bc2"]
                xr_t = xr[min(t, T - 1) % 2]

                for p in range(NPAIR):
                    pc = slice(p * 512, (p + 1) * 512)        # packed cols
                    tc_cols = slice(p * 1024, p * 1024 + 512)  # real cols top
                    bc_cols = slice(p * 1024 + 512, (p + 1) * 1024)  # real bottom

                    # ---- attention logits for both chunks of the pair.
                    # Group g_w (holding the slot written last step) goes last
                    # so the other groups' matmuls don't wait on the q copy.
                    gorder = [g for g in range(3) if g != g_w] + [g_w]
                    lg2 = pp_lg.tile([96, 1024], F32, tag="lg")
                    for half, rcols in ((0, tc_cols), (1, bc_cols)):
                        hs = slice(half * 512, (half + 1) * 512)
                        for gi, g in enumerate(gorder):
                            off = (r * 3 + g) * 96
                            nc.tensor.matmul(
                                lg2[:, hs], c["msk"][:, off:off + 96], qb[g][:, rcols],
                                start=(gi == 0), stop=(gi == 2),
                            )
                    ex2 = sp.tile([96, 1024], BF16, tag="ex")
                    nc.scalar.activation(ex2[:], lg2[:], AF.Exp, bias=c["lbias"][:, 0:1])
                    fu2 = pp_fu.tile([128, 512], F32, tag="fu")
                    su2 = pp_su.tile([128, 512], F32, tag="su")
                    nc.tensor.matmul(fu2[0:64, :], c["fmean"][:], ex2[:, 0:512],
                                     start=True, stop=True, tile_position=(0, 0),
                                     skip_group_check=True)
                    nc.tensor.matmul(fu2[64:128, :], c["fmean"][:], ex2[:, 512:1024],
                                     start=True, stop=True, tile_position=(0, 64),
                                     skip_group_check=True)
                    nc.tensor.matmul(su2[0:64, :], c["fsum"][:], ex2[:, 0:512],
                                     start=True, stop=True, tile_position=(0, 0),
                                     skip_group_check=True)
                    nc.tensor.matmul(su2[64:128, :], c["fsum"][:], ex2[:, 512:1024],
                                     start=True, stop=True, tile_position=(0, 64),
                                     skip_group_check=True)
                    rt2 = sp.tile([128, 512], F32, tag="rt")
                    nc.vector.reciprocal_approx_fast(rt2[:], su2[:])
                    fn2 = sp.tile([128, 512], FP16, tag="fn")
                    nc.vector.tensor_mul(fn2[:], fu2[:], rt2[:])

                    # ---- gates z, r (pair-packed, block-diag weights)
                    z2 = pp_z.tile([128, 512], F32, tag="z2")
                    r2 = pp_r.tile([128, 512], F32, tag="r2")
                    nc.tensor.matmul(z2[:], zbd_t[:], h2[:, pc],
                                     start=True, stop=not enc, skip_group_check=True)
                    if enc:
                        nc.tensor.matmul(z2[:], c["xzw"][:], xr_t[:, pc],
                                         start=False, stop=True, tile_position=(0, 0),
                                         skip_group_check=True)
                    nc.tensor.matmul(r2[:], rbd_t[:], h2[:, pc],
                                     start=True, stop=not enc, skip_group_check=True)
                    if enc:
                        nc.tensor.matmul(r2[:], c["xrw"][:], xr_t[:, pc],
                                         start=False, stop=True, tile_position=(0, 0),
                                         skip_group_check=True)
                    tzr2 = sp.tile([128, 1024], FP16, tag="tzr")
                    nc.scalar.activation(tzr2[:, 0:512], z2[:], AF.Tanh, bias=bz_t[:, 0:1])
                    nc.scalar.activation(tzr2[:, 512:1024], r2[:], AF.Tanh, bias=br_t[:, 0:1])
                    # z gate: sigmoid(x) = 0.5 + 0.5*tanh(x/2) (weights pre-halved)
                    zs2 = sp.tile([128, 512], FP16, tag="zs")
                    nc.gpsimd.tensor_scalar(zs2[:], tzr2[:, 0:512], 0.5, 0.5, OP.mult, OP.add)
                    # r gate folded into cw weights: Wch^T(r.h) = (.5Wch)^T h + (.5Wch)^T (tr.h)
                    trh2 = sp.tile([128, 512], FP16, tag="trh")
                    nc.vector.tensor_mul(trh2[:], tzr2[:, 512:1024], h2[:, pc])

                    # ---- candidate hc
                    acc2 = pp_acc.tile([128, 512], F32, tag="acc")
                    nc.tensor.matmul(acc2[:], cbd_t[:], h2[:, pc],
                                     start=True, stop=False, skip_group_check=True)
                    nc.tensor.matmul(acc2[:], c["cbd05"][:], trh2[:],
                                     start=False, stop=False, skip_group_check=True)
                    if enc:
                        nc.tensor.matmul(acc2[:], c["xcw"][:], xr_t[:, pc],
                                         start=False, stop=False, tile_position=(0, 0),
                                         skip_group_check=True)
                    # hypernet: block-diag node-pairs (top node k | bottom node
                    # 16+k) -> one [128,128,32] matmul per k
                    for k in range(16):
                        kp = 16 * p + k
                        nc.tensor.matmul(
                            acc2[:, k * 32:(k + 1) * 32],
                            c["nswbd"][:, kp * 128:(kp + 1) * 128],
                            fn2[:, k * 32:(k + 1) * 32],
                            start=False, stop=(k == 15), tile_position=(0, 0),
                            skip_group_check=True,
                        )
                    hc2 = sp.tile([128, 512], FP16, tag="hc")
                    nc.scalar.activation(hc2[:], acc2[:], AF.Tanh, bias=bc_t[:, 0:1])

                    # ---- GRU update: h += (0.5 + 0.5 tz) * (hc - h)
                    d2 = sp.tile([128, 512], FP16, tag="d2")
                    nc.vector.tensor_sub(d2[:], hc2[:], h2[:, pc])
                    nc.vector.tensor_mul(d2[:], zs2[:], d2[:])
                    nc.vector.tensor_add(h2[:, pc], h2[:, pc], d2[:])

                    # ---- q cache update (one block-diag matmul + 2 shift copies)
                    qp2 = pp_qy.tile([64, 512], F32, tag="qy")
                    nc.tensor.matmul(qp2[:], c["qw128"][:], h2[:, pc],
                                     start=True, stop=True, skip_group_check=True)
                    nc.vector.tensor_copy(qb[g_w][row_w:row_w + 32, tc_cols], qp2[0:32, :])
                    nc.scalar.activation(qb[g_w][row_w:row_w + 32, bc_cols], qp2[32:64, :],
                                         AF.Identity)

                    # ---- decode output: DMA straight from PSUM
                    if t >= T:
                        y2 = pp_qy.tile([2, 512], F32, tag="qy")
                        nc.tensor.matmul(y2[:], c["ow128"][:], h2[:, pc],
                                         start=True, stop=True, skip_group_check=True)
                        nc.scalar.activation(ystage[:, pc], y2[:], AF.Identity,
                                             bias=c["bo2"][:, 0:1])

                # ---- step epilogue DMAs
                if t < T - 1:
                    nc.sync.dma_start(xr[(t + 1) % 2][:],
                                      d["xsrc"][2 * (t + 1):2 * (t + 2), :])
                if t >= T:
                    dstep = t - T
                    nc.sync.dma_start(ys_d[2 * dstep:2 * (dstep + 1), :], ystage[:])
    nc.compile()
    return nc


def precompute(inp):
    f32 = np.float32
    lm = np.asarray(inp["local_mem"], f32)
    gm = np.asarray(inp["global_mem"], f32)
    Wq = np.asarray(inp["Wq"], f32)
    bq = np.asarray(inp["bq"], f32)
    node_emb = np.asarray(inp["node_emb"], f32)
    wp = np.asarray(inp["weight_pool"], f32)
    Wz = np.asarray(inp["Wz"], f32); bz = np.asarray(inp["bz"], f32)
    Wr = np.asarray(inp["Wr"], f32); br = np.asarray(inp["br"], f32)
    Wc = np.asarray(inp["Wc"], f32); bc = np.asarray(inp["bc"], f32)
    Wo = np.asarray(inp["Wo"], f32); bo = np.asarray(inp["bo"], f32)

    c = {}
    # per-node hypernet weights, duplicated on both partition halves
    c["nsw_full"] = np.einsum("nd,dfh->nfh", node_emb, wp).astype(f32)  # [N,2P,H]
    # memory rotation stacks (identical to baseline)
    memsl = np.concatenate([lm.transpose(2, 0, 1), gm.transpose(2, 0, 1)], axis=1)  # [P,96,S]
    ms = np.zeros((128, S, 3, 96), f32)
    for rr in range(S):
        for g in range(3):
            for i in range(4):
                s = (4 * g + i - rr) % S
                ms[32 * i:32 * (i + 1), rr, g, :] = memsl[:, :, s]
    c["msk"] = ms.reshape(128, S * 3 * 96)
    lmean, gmean = lm.mean(axis=1), gm.mean(axis=1)
    fm = np.zeros((96, 64), f32)
    fm[:ML, :P] = lmean
    fm[ML:, P:2 * P] = gmean
    c["fmean"] = fm
    fs = np.zeros((96, 64), f32)
    fs[:ML, :P] = 1.0
    fs[ML:, P:2 * P] = 1.0
    c["fsum"] = fs

    def blockdiag(w):  # w [64,64] as lhsT [k,i] -> [128,128]
        o = np.zeros((128, 128), f32)
        o[0:64, 0:64] = w
        o[64:128, 64:128] = w
        return o

    Wzh, Wzx = Wz[1:], Wz[0]   # [64,64], [64]
    Wrh, Wrx = Wr[1:], Wr[0]
    Wch, Wcx = Wc[1:], Wc[0]
    wo = Wo[:, 0]              # [64]
    c["zbd"] = blockdiag(0.5 * Wzh)
    c["rbd"] = blockdiag(0.5 * Wrh)
    c["cbd05"] = blockdiag(0.5 * Wch)
    c["zbd_dec"] = blockdiag(0.5 * (Wzh + np.outer(wo, Wzx)))
    c["rbd_dec"] = blockdiag(0.5 * (Wrh + np.outer(wo, Wrx)))
    c["cbd_dec"] = blockdiag(0.5 * Wch + np.outer(wo, Wcx))

    def xw2(wx, scale):  # [2,128] rank-2 x weights
        o = np.zeros((2, 128), f32)
        o[0, 0:64] = scale * wx
        o[1, 64:128] = scale * wx
        return o

    c["xzw"] = xw2(Wzx, 0.5)
    c["xrw"] = xw2(Wrx, 0.5)
    c["xcw"] = xw2(Wcx, 1.0)

    q128 = np.zeros((128, 64), f32)
    q128[0:64, 0:32] = Wq
    q128[64:128, 32:64] = Wq
    c["qw128"] = q128
    o128 = np.zeros((128, 2), f32)
    o128[0:64, 0] = wo
    o128[64:128, 1] = wo
    c["ow128"] = o128

    # bq folded into a constant logit bias: lbias[m] = bq . sum_s mem[m, s, :]
    lb = np.zeros((96, 1), f32)
    lb[:ML, 0] = np.einsum("msp,p->m", lm, bq)
    lb[ML:, 0] = np.einsum("msp,p->m", gm, bq)
    c["lbias"] = lb
    c["bz05"] = np.tile(0.5 * bz, 2).reshape(128, 1)
    c["br05"] = np.tile(0.5 * br, 2).reshape(128, 1)
    c["bz_dec"] = np.tile(0.5 * (bz + Wzx * bo[0]), 2).reshape(128, 1)
    c["br_dec"] = np.tile(0.5 * (br + Wrx * bo[0]), 2).reshape(128, 1)
    c["bc2"] = np.tile(bc, 2).reshape(128, 1)
    c["bc_dec"] = np.tile(bc + Wcx * bo[0], 2).reshape(128, 1)
    c["bo2"] = np.full((2, 1), bo[0], f32)
    return c


def _f16(a):
    return np.ascontiguousarray(a).astype(np.float16)


def _bf16(a):
    import ml_dtypes
    return np.ascontiguousarray(a).astype(ml_dtypes.bfloat16)


def make_in_maps(inp):
    c = precompute(inp)
    src = np.asarray(inp["source"], np.float32)
    shared = {
        "msk": _f16(c["msk"]),
        "fmean": _bf16(c["fmean"]), "fsum": _bf16(c["fsum"]),
        "zbd": _f16(c["zbd"]), "rbd": _f16(c["rbd"]),
        "zbd_dec": _f16(c["zbd_dec"]), "rbd_dec": _f16(c["rbd_dec"]),
        "cbd05": _f16(c["cbd05"]), "cbd_dec": _f16(c["cbd_dec"]),
        "xzw": _f16(c["xzw"]), "xrw": _f16(c["xrw"]), "xcw": _f16(c["xcw"]),
        "qw128": _f16(c["qw128"]), "ow128": _f16(c["ow128"]),
        "lbias": c["lbias"], "bz05": c["bz05"], "br05": c["br05"],
        "bz_dec": c["bz_dec"], "br_dec": c["br_dec"],
        "bc2": c["bc2"], "bc_dec": c["bc_dec"], "bo2": c["bo2"],
    }
    in_maps = []
    for core in range(NCORES):
        nodes = slice(core * NL, (core + 1) * NL)
        # real col = n_local*32 + b ; xsrc packed [T, 2, 1024]:
        # [t, half, p*512 + j] = x_t[real col p*1024 + half*512 + j]
        xs = src[:, :, nodes, 0].transpose(1, 2, 0).reshape(T, NB)  # [T, real]
        xs = xs.reshape(T, 2, 2, 512).transpose(0, 2, 1, 3).reshape(T * 2, NB // 2)
        nswc = c["nsw_full"][nodes]  # [NL, 64, 64] (f, h)
        # block-diag node-pair lhsT: per pair p, k: [0:64,0:64]=nsw[32p+k],
        # [64:128,64:128]=nsw[32p+16+k]; col block index kp = 16p+k
        nswbd = np.zeros((128, (NL // 2) * 128), np.float32)
        for p in range(2):
            for k in range(16):
                kp = 16 * p + k
                nswbd[0:64, kp * 128:kp * 128 + 64] = nswc[32 * p + k]
                nswbd[64:128, kp * 128 + 64:(kp + 1) * 128] = nswc[32 * p + 16 + k]
        in_maps.append(dict(shared, xsrc=_f16(xs), nswbd=_f16(nswbd)))
    return in_maps


def assemble(results):
    out = np.zeros((B, HORIZON, N, OUT), np.float32)
    for core in range(NCORES):
        nodes = slice(core * NL, (core + 1) * NL)
        ys = np.asarray(results[core]["ys"], np.float32)  # [HORIZON, 2, 1024]
        # real col = p*1024 + half*512 + j
        ysr = ys.reshape(HORIZON, 2, 2, 512).transpose(0, 2, 1, 3).reshape(HORIZON, NB)
        out[:, :, nodes, 0] = ysr.reshape(HORIZON, NL, B).transpose(2, 0, 1)
    return out


_NC_CACHE = {}


def kernel(**inputs):
    if "nc" not in _NC_CACHE:
        _NC_CACHE["nc"] = build_nc()
    nc = _NC_CACHE["nc"]
    in_maps = make_in_maps(inputs)
    res = bass_utils.run_bass_kernel_spmd(nc, in_maps, core_ids=list(range(NCORES)))
    return assemble(res.results)
